# revision 11
# baseline (speedup 1.0000x reference)
"""Locally-connected 2D conv (unshared weights), VALID, stride 2 — Trainium2 Bass kernel.

Problem (hardcoded):
  x:       (16, 32, 113, 113) f32
  weights: (56, 56, 32, 3, 3, 64) f32   (H_out, W_out, C_in, kh, kw, C_out)
  bias:    (56, 56, 64) f32
  out:     (16, 64, 56, 56) f32
  out[b,o,u,v] = sum_{c,q,r} x[b,c,2u+q,2v+r] * weights[u,v,c,q,r,o] + bias[u,v,o]

Sharding: H_out split across 8 cores (7 output rows each); each core reads only
its 1/8 of the weight tensor (the dominant traffic).

v2 (bf16): matmul operands are bf16 (host-rounded). The fp32 PE path runs
multi-pass (measured 106.5ns per LDWEIGHTS+MATMUL pair, 2x4-pass); bf16 is
single-pass AND halves HBM bytes — this kernel is DMA-footprint-bound at
~18.4MB/core against a measured ~224GB/s aggregate SDMA ceiling (16 engines x
~14GB/s). Accuracy: bf16 w+x gives 2.5e-3 max rel err (gate 2e-2).

Layouts (host-packed):
  xp[p=(q,c), t, b, w] = x[b, c, 2*(u0+t)+q, w]  bf16 [96,7,16,113]
    - only the 7 rows j=2t are ever read by matmuls (q-shift baked into
      partitions), so only 2.4MB instead of the 9MB f32 tripled layout.
  wp[t, ch, p=(q,c), (v r o)]                    bf16 [7,2,96,5376]
  bp[o, t*56+v]                                  f32  [64,392]
  y [o, t, b, v]                                 bf16 [64,7,16,56] (o-major so
    per-u DMA is 64 descriptors x 1792B contiguous; host transposes back)

Per-core compute unchanged from v1: per (u, ch, v, r):
  psum(o=64, b=16) += W[k=(q,c), r*64+o].T @ X[k, b], one PSUM accumulation
group per 28-v chunk; DVE adds bias during PSUM->SBUF (casting to bf16).

DMA: two HWDGE rings (SP + Act), ~9MB each, x tiles interleaved by need so
u=0 can start ~5us in (first weight chunk split in 4 for early start).
"""

import numpy as np
import ml_dtypes

B = 16
C_IN = 32
C_OUT = 64
H_OUT = 56
W_OUT = 56
KK = 3
STRIDE = 2
H_IN = 113

N_CORES = 8
U_PER = H_OUT // N_CORES          # 7 output rows per core
VCHUNK = 28                       # output cols per PSUM bank chunk
NCHUNK = W_OUT // VCHUNK          # 2 chunks per u
WFREE = VCHUNK * KK * C_OUT       # weight chunk free size (5376)
KPART = C_IN * KK                 # 96 contraction partitions (q,c)

_CACHE = {}


def _build():
    import concourse.mybir as mybir
    from concourse import bacc
    from concourse.tile import TileContext

    f32 = mybir.dt.float32
    bf16 = mybir.dt.bfloat16
    nc = bacc.Bacc("TRN2", target_bir_lowering=False, debug=False,
                   num_devices=N_CORES)
    xp_in = nc.dram_tensor("xp", [KPART, U_PER, B, H_IN], bf16,
                           kind="ExternalInput").ap()
    wp_in = nc.dram_tensor("wp", [U_PER, NCHUNK, KPART, WFREE], bf16,
                           kind="ExternalInput").ap()
    bp_in = nc.dram_tensor("bp", [C_OUT, U_PER * W_OUT], f32,
                           kind="ExternalInput").ap()
    y_out = nc.dram_tensor("y", [C_OUT, U_PER, B, W_OUT], bf16,
                           kind="ExternalOutput").ap()

    with TileContext(nc) as tc:
        with tc.tile_pool(name="persist", bufs=1) as perpool, \
             tc.tile_pool(name="xpool", bufs=4) as xpool, \
             tc.tile_pool(name="wpool", bufs=8) as wpool, \
             tc.tile_pool(name="pspool", bufs=4, space="PSUM") as pspool:

            bt = perpool.tile([C_OUT, U_PER * W_OUT], f32, name="bt")
            oa = perpool.tile([C_OUT, U_PER, B, W_OUT], bf16, name="oa")

            # Ring plan: the two HWDGE rings carry ONLY weights (7.2MB each,
            # even chunks -> Act, odd -> SP) so the weight stream is never
            # stuck behind x. x/bias/y ride the gpsimd sw-DGE ring (Q0
            # streams at the same per-engine rate). xt0 jumps the scalar
            # queue ahead of w0 since nothing can start without it; later x
            # tiles are throttled by the 4-deep xpool rotation so they don't
            # steal startup bandwidth from the critical first weight bytes.
            xts = []
            xt0 = xpool.tile([KPART, B, H_IN], bf16, name="xt")
            nc.scalar.dma_start(out=xt0[:], in_=xp_in[:, 0])
            xts.append(xt0)
            nc.gpsimd.dma_start(out=bt[:], in_=bp_in[:])

            QV = WFREE // 4  # 1344 (aligned to 192: 7 v's per quarter)
            for u in range(U_PER):
                if u + 1 < U_PER:
                    xt = xpool.tile([KPART, B, H_IN], bf16, name="xt")
                    nc.gpsimd.dma_start(out=xt[:], in_=xp_in[:, u + 1])
                    xts.append(xt)
                for ch in range(NCHUNK):
                    k = u * NCHUNK + ch
                    wt = wpool.tile([KPART, WFREE], bf16, name="wt")
                    weng = nc.scalar if k % 2 == 0 else nc.sync
                    if k == 0:
                        # first chunk split across BOTH hwdge rings so the
                        # critical startup bytes get 2 queues' bandwidth
                        for s in range(4):
                            eng = nc.scalar if s % 2 == 0 else nc.sync
                            eng.dma_start(
                                out=wt[:, s * QV:(s + 1) * QV],
                                in_=wp_in[0, 0, :, s * QV:(s + 1) * QV])
                    else:
                        weng.dma_start(out=wt[:], in_=wp_in[u, ch])
                    wt3 = wt.rearrange("p (v ro) -> p v ro", v=VCHUNK)
                    ps = pspool.tile([C_OUT, VCHUNK * B], f32, name="ps")
                    for vl in range(VCHUNK):
                        v = ch * VCHUNK + vl
                        for r in range(KK):
                            lhsT = wt3[:, vl:vl + 1,
                                       r * C_OUT:(r + 1) * C_OUT]
                            col = STRIDE * v + r
                            rhs = xts[u][:, :, col:col + 1]
                            nc.tensor.matmul(
                                ps[:, vl * B:(vl + 1) * B], lhsT, rhs,
                                start=(vl == 0 and r == 0),
                                stop=(vl == VCHUNK - 1 and r == KK - 1),
                            )
                    ps3 = ps.rearrange("p (v b) -> p b v", v=VCHUNK)
                    v0 = ch * VCHUNK
                    uv = u * W_OUT + v0
                    bslice = bt[:, uv:uv + VCHUNK].unsqueeze(1).broadcast_to(
                        [C_OUT, B, VCHUNK])
                    nc.vector.tensor_add(
                        oa[:, u, :, v0:v0 + VCHUNK], ps3, bslice)
                # y on the gpsimd (sw-DGE) ring so its dependency wait can't
                # block the weight-trigger queues; the last one goes via the
                # by-then-idle Act ring (hw descriptor gen, shorter tail)
                yeng = nc.scalar if u == U_PER - 1 else nc.gpsimd
                yeng.dma_start(out=y_out[:, u], in_=oa[:, u])

    nc.compile()
    return nc


def _get_nc():
    if "nc" not in _CACHE:
        _CACHE["nc"] = _build()
    return _CACHE["nc"]


def _pack_core(x, weights, bias, i):
    bf16 = ml_dtypes.bfloat16
    u0 = i * U_PER
    # x': (96, 7, 16, 113); partition p=(q,c) slot t holds x[b, c, 2(u0+t)+q]
    xs = x[:, :, STRIDE * u0:STRIDE * u0 + 2 * (U_PER - 1) + KK, :]
    xq = np.stack([xs[:, :, q:q + 2 * U_PER - 1:2, :].transpose(1, 2, 0, 3)
                   for q in range(KK)], axis=0)      # (q, c, t, b, w)
    xp = np.ascontiguousarray(
        xq.reshape(KPART, U_PER, B, H_IN).astype(bf16))

    # w': (7, 2, 96, 5376); p = q*32+c, free (v, r, o)
    ws = weights[u0:u0 + U_PER].reshape(U_PER, NCHUNK, VCHUNK, C_IN, KK, KK,
                                        C_OUT)
    ws = ws.transpose(0, 1, 4, 3, 2, 5, 6)           # (u, ch, q, c, v, r, o)
    wp = np.ascontiguousarray(
        ws.reshape(U_PER, NCHUNK, KPART, WFREE).astype(bf16))

    # b': (64, 392): bp[o, t*56+v]
    bp = np.ascontiguousarray(
        bias[u0:u0 + U_PER].reshape(U_PER * W_OUT, C_OUT).T)
    return {"xp": xp, "wp": wp, "bp": bp}


def kernel(x, weights, bias, _trace=False, _tmpdir=None):
    from concourse.bass_utils import run_bass_kernel_spmd

    x = np.ascontiguousarray(x, dtype=np.float32)
    weights = np.ascontiguousarray(weights, dtype=np.float32)
    bias = np.ascontiguousarray(bias, dtype=np.float32)

    nc = _get_nc()
    core_ids = list(range(N_CORES))
    in_maps = [_pack_core(x, weights, bias, i) for i in core_ids]
    res = run_bass_kernel_spmd(nc, in_maps, core_ids, trace=_trace,
                               tmpdir=_tmpdir)
    # y[o, t, b, v] bf16 -> (b, o, u, v) f32, concat cores along u
    out = np.concatenate(
        [np.asarray(res.results[i]["y"]).astype(np.float32)
         .transpose(2, 0, 1, 3) for i in core_ids], axis=2)
    if _trace:
        _CACHE["last_result"] = res
    return out
